# revision 9
# baseline (speedup 1.0000x reference)
"""Multi-head attention (B=2, S=2048, D=1024, H=16, hd=64) with RoPE on 8 TRN2
NeuronCores.

Sharding: 2 batches x 4 head-groups. Core c handles batch c//4, heads
[4*(c%4), 4*(c%4)+4). Each core computes Q/K/V projections for its heads from
the full sequence, RoPE, then streamed attention per head-pair: scores as two
concurrent K=64 row-tiles, exp on the scalar engine (fp16 out), attnV as two
concurrent M=64 column-tiles of the PE array, and softmax row-sums from a DVE
accumulation of the exp tiles (scaled by exp(mask)) reduced by a ones-vector
matmul. The output projection is chunked by 512-row blocks as PE filler work
inside the next block's attention iterations, and each block is
ReduceScattered over the batch's 4-core group in two 256-row sub-chunks so
nearly all collective time overlaps attention. Core g of a group keeps rows
[256*ch + 64*g, +64) of sub-chunk ch; the host reassembles and adds the
(wo + wv@wo) bias.

Layout notes:
- x is uploaded pre-transposed (xT [D, S]) so it serves both as matmul rhs for
  Q^T/K^T production and as lhsT for V production.
- Q^T/K^T rows within each head are permuted to (d0,d32,d1,d33,...) so the
  RoPE partner lives in the adjacent partition; a stream_shuffle with the
  pair-swap mask plus two multiplies by host-precomputed cos/sin tables
  implements the rotation with all operands partition-aligned. The score
  matmul contracts over the permuted axis, which is permutation-invariant as
  long as Q and K share the ordering.
- The attention mask enters as exp(mask[k]): multiplied into V's rows for the
  numerator and into the exp tiles inside the row-sum accumulation, which is
  exact.
- Collectives live alone on the gpsimd queue; every DMA another engine later
  depends on is kept off that queue so a ReduceScatter wait can never head-block
  the attention pipeline.
"""

import numpy as np
import ml_dtypes

import concourse.bass as bass
import concourse.mybir as mybir
from concourse import bacc, bass_utils
import concourse.tile as tile

B, S, DIM, HEADS, HD = 2, 2048, 1024, 16, 64
HPC = HEADS // 4          # heads per core = 4
P = 128
NKC = DIM // P            # 8 contraction chunks for projections
NSC = S // P              # 16 sequence chunks of 128
NQT = S // 512            # 4 q tiles of 512
SQ = S // 4               # 512-row output slice per core
fp32 = mybir.dt.float32
bf16 = mybir.dt.bfloat16
fp16 = mybir.dt.float16

_CACHE = {}


def _build():
    nc = bacc.Bacc("TRN2", target_bir_lowering=False, debug=False, num_devices=8)

    xT = nc.dram_tensor("xT", [DIM, S], bf16, kind="ExternalInput")
    wq = nc.dram_tensor("wq", [DIM, HPC * HD], bf16, kind="ExternalInput")
    wk = nc.dram_tensor("wk", [DIM, HPC * HD], bf16, kind="ExternalInput")
    wv = nc.dram_tensor("wv", [DIM, HPC * HD], bf16, kind="ExternalInput")
    wo = nc.dram_tensor("wo", [HPC * HD, DIM], bf16, kind="ExternalInput")
    trigA = nc.dram_tensor("trigA", [P, S], bf16, kind="ExternalInput")
    trigB = nc.dram_tensor("trigB", [P, S], bf16, kind="ExternalInput")
    qbias = nc.dram_tensor("qbias", [P, 2], fp32, kind="ExternalInput")
    kbias = nc.dram_tensor("kbias", [P, 2], fp32, kind="ExternalInput")
    em = nc.dram_tensor("em", [P, NSC], fp32, kind="ExternalInput")
    out = nc.dram_tensor("out", [SQ, DIM], bf16, kind="ExternalOutput")

    SWAP_MASK = [i ^ 1 for i in range(32)]

    with tile.TileContext(nc) as tc:
        with (
            tc.tile_pool(name="const", bufs=1) as const,
            tc.tile_pool(name="work", bufs=3) as work,
            tc.tile_pool(name="attp", bufs=6) as attp,
            tc.tile_pool(name="accp", bufs=2) as accp,
            tc.tile_pool(name="stun", bufs=10) as stun,
            tc.tile_pool(name="ps_proj", bufs=2, space="PSUM") as ps_proj,
            tc.tile_pool(name="ps_sT", bufs=2, space="PSUM") as ps_sT,
            tc.tile_pool(name="ps_oT", bufs=2, space="PSUM") as ps_oT,
            tc.tile_pool(name="dram", bufs=1, space="DRAM") as dram,
            tc.tile_pool(name="dram_rc", bufs=3, space="DRAM") as dram_rc,
        ):
            # ---- load constants / inputs into SBUF ----
            # xT is split per kc chunk and spread over the sync and scalar
            # HWDGE queues (wq first on scalar so the kc=0 matmuls start
            # early); the gpsimd SWDGE queue carries only the later-needed
            # weights so the collectives it also hosts never gate anything.
            xT_sb = const.tile([P, NKC, S], bf16)
            xT_r = xT.rearrange("(c p) s -> p c s", p=P)
            wq_sb = const.tile([P, NKC, HPC * HD], bf16)
            wk_sb = const.tile([P, NKC, HPC * HD], bf16)
            wv_sb = const.tile([P, NKC, HPC * HD], bf16)
            wq_r = wq.rearrange("(c p) m -> p c m", p=P)
            wk_r = wk.rearrange("(c p) m -> p c m", p=P)
            wv_r = wv.rearrange("(c p) m -> p c m", p=P)
            nc.scalar.dma_start(wq_sb[:, 0:4, :], wq_r[:, 0:4, :])
            nc.sync.dma_start(xT_sb[:, 0, :], xT_r[:, 0, :])
            nc.scalar.dma_start(xT_sb[:, 1, :], xT_r[:, 1, :])
            nc.sync.dma_start(xT_sb[:, 2, :], xT_r[:, 2, :])
            nc.scalar.dma_start(wq_sb[:, 4:8, :], wq_r[:, 4:8, :])
            nc.sync.dma_start(xT_sb[:, 4, :], xT_r[:, 4, :])
            nc.scalar.dma_start(xT_sb[:, 3, :], xT_r[:, 3, :])
            nc.gpsimd.dma_start(wk_sb[:, 0:4, :], wk_r[:, 0:4, :])
            nc.sync.dma_start(xT_sb[:, 6, :], xT_r[:, 6, :])
            nc.scalar.dma_start(xT_sb[:, 5, :], xT_r[:, 5, :])
            nc.gpsimd.dma_start(wk_sb[:, 4:8, :], wk_r[:, 4:8, :])
            nc.scalar.dma_start(xT_sb[:, 7, :], xT_r[:, 7, :])
            qb_sb = const.tile([P, 2], fp32)
            nc.gpsimd.dma_start(qb_sb[:], qbias[:])
            kb_sb = const.tile([P, 2], fp32)
            nc.gpsimd.dma_start(kb_sb[:], kbias[:])
            em_sb = const.tile([P, NSC], fp32)
            nc.gpsimd.dma_start(em_sb[:], em[:])
            trigA_sb = const.tile([P, S], bf16)
            nc.scalar.dma_start(trigA_sb[:], trigA[:])
            trigB_sb = const.tile([P, S], bf16)
            nc.sync.dma_start(trigB_sb[:], trigB[:])
            nc.gpsimd.dma_start(wv_sb[:, 0:4, :], wv_r[:, 0:4, :])
            nc.gpsimd.dma_start(wv_sb[:, 4:8, :], wv_r[:, 4:8, :])
            wo_sb = const.tile([P, 2, DIM], bf16)
            nc.scalar.dma_start(wo_sb[:], wo.rearrange("(c p) m -> p c m", p=P))

            warm_in = dram.tile([P, 4], fp32, name="warm_in")
            warm_out = dram.tile([P, 4], fp32, name="warm_out")
            wz = work.tile([P, 4], fp32, tag="wz", name="wz")
            nc.vector.memset(wz[:], 0.0)
            nc.gpsimd.dma_start(warm_in[:], wz[:])
            nc.gpsimd.collective_compute(
                "AllReduce", mybir.AluOpType.add,
                replica_groups=[[0, 1, 2, 3], [4, 5, 6, 7]],
                ins=[warm_in.opt()], outs=[warm_out.opt()],
            )

            QT_rot = const.tile([P, 2, S], bf16)   # heads 0,1 | 2,3 stacked
            KT_rot = const.tile([P, 2, S], bf16)
            V_sb = const.tile([P, NSC, HPC * HD], fp16)  # em-scaled V
            ones_sb = const.tile([P, 1], fp16)
            nc.vector.memset(ones_sb[:], 1.0)

            # ---- Q^T / K^T projections + RoPE ----
            # kc-outer so the stationary weight chunk is loaded once per kc
            # and streams the s-chunks. Only the cq=0 chunk (heads 0,1) is
            # produced up front; cq=1 matmuls are interleaved into head-pair
            # 0's attention iterations, where the PE has slack.
            def rope_chain(pss_sc, b_sb, dst, cq, sc, on_scalar):
                q_sb = work.tile([P, 512], bf16, tag="q_sb",
                                 name=f"q_sb_{cq}_{sc}")
                if on_scalar:
                    nc.scalar.add(q_sb[:], pss_sc[:], b_sb[:, cq:cq + 1])
                else:
                    nc.vector.tensor_scalar_add(
                        q_sb[:], pss_sc[:], b_sb[:, cq:cq + 1])
                q_sw = work.tile([P, 512], bf16, tag="q_sw",
                                 name=f"q_sw_{cq}_{sc}")
                nc.vector.stream_shuffle(q_sw[:], q_sb[:], SWAP_MASK)
                p1 = work.tile([P, 512], bf16, tag="p1", name=f"p1_{cq}_{sc}")
                nc.vector.tensor_mul(
                    p1[:], q_sb[:], trigA_sb[:, sc * 512:(sc + 1) * 512])
                p2 = work.tile([P, 512], bf16, tag="p2", name=f"p2_{cq}_{sc}")
                nc.vector.tensor_mul(
                    p2[:], q_sw[:], trigB_sb[:, sc * 512:(sc + 1) * 512])
                nc.vector.tensor_add(
                    dst[:, cq, sc * 512:(sc + 1) * 512], p1[:], p2[:])

            for w_sb, b_sb, dst in ((wq_sb, qb_sb, QT_rot), (wk_sb, kb_sb, KT_rot)):
                pss = []
                for sc in range(4):
                    pool = ps_proj if sc < 2 else ps_sT
                    tag = "proj" if sc < 2 else "sT"
                    pss.append(pool.tile([P, 512], fp32, tag=tag,
                                         name=f"pss_0_{sc}"))
                for kc in range(NKC):
                    for sc in range(4):
                        nc.tensor.matmul(
                            pss[sc][:],
                            w_sb[:, kc, 0:P],
                            xT_sb[:, kc, sc * 512:(sc + 1) * 512],
                            start=(kc == 0), stop=(kc == NKC - 1),
                        )
                for sc in range(4):
                    rope_chain(pss[sc], b_sb, dst, 0, sc, on_scalar=True)

            # deferred cq=1 projection groups: (weights, bias, dst, sc pair)
            defer_groups = [
                (wq_sb, qb_sb, QT_rot, (0, 1)),
                (wq_sb, qb_sb, QT_rot, (2, 3)),
                (wk_sb, kb_sb, KT_rot, (0, 1)),
                (wk_sb, kb_sb, KT_rot, (2, 3)),
            ]
            defer_pss = {}

            def defer_step(it):
                gi, m = divmod(it, 16)
                w_sb, b_sb, dst, scs = defer_groups[gi]
                if m == 0:
                    defer_pss[gi] = [
                        ps_proj.tile([P, 512], fp32, tag="proj",
                                     name=f"ip_{gi}_{k}")
                        for k in range(2)]
                kc, si = divmod(m, 2)
                nc.tensor.matmul(
                    defer_pss[gi][si][:],
                    w_sb[:, kc, P:2 * P],
                    xT_sb[:, kc, scs[si] * 512:(scs[si] + 1) * 512],
                    start=(kc == 0), stop=(kc == NKC - 1),
                )
                if m == 15:
                    for si in range(2):
                        rope_chain(defer_pss[gi][si], b_sb, dst, 1, scs[si],
                                   on_scalar=False)

            # ---- V projection (natural layout, scaled by exp(mask)) ----
            for sc in range(NSC):
                ps = ps_proj.tile([P, HPC * HD], fp32, tag="proj")
                for kc in range(NKC):
                    nc.tensor.matmul(
                        ps[:],
                        xT_sb[:, kc, sc * P:(sc + 1) * P],
                        wv_sb[:, kc, :],
                        start=(kc == 0), stop=(kc == NKC - 1),
                    )
                nc.vector.tensor_scalar_mul(
                    V_sb[:, sc, :], ps[:], em_sb[:, sc:sc + 1])

            # ---- chunked output projection + ReduceScatter plumbing ----
            # oproj(qt) contracts the core's 4 heads for rows [512qt, +512)
            # and is emitted as PE filler work inside (hp=1, qt+1)'s
            # attention iterations; each 256-row half is ReduceScattered as
            # soon as its partials are staged, overlapping the remaining
            # attention. Sub-chunk ch rows land on group rank g as out rows
            # [64ch, +64) (host reassembles).
            oT_norm = const.tile([P, 2, S], bf16)   # normalized o^T, heads packed
            cc_in = [dram.tile([256, DIM], bf16, name=f"cc_in_{ch}")
                     for ch in range(2 * NQT)]
            cc_out = [dram.tile([64, DIM], bf16, name=f"cc_out_{ch}")
                      for ch in range(2 * NQT)]
            oproj_sb = {}

            def oproj_mm(qt, m):
                # one matmul of oproj(qt): m in [0,16)
                qs, r = divmod(m, 4)        # 4 row-chunks of 128
                dc, c = divmod(r, 2)        # 2 output halves x 2 contraction
                if m == 0:
                    oproj_sb[qt] = work.tile([P, 4, DIM], bf16, tag="o_sb",
                                             name=f"o_sb_{qt}")
                if c == 0:
                    oproj_sb[(qt, 'ps', dc)] = ps_proj.tile(
                        [P, 512], fp32, tag="proj", name=f"op_{qt}_{qs}_{dc}")
                ps = oproj_sb[(qt, 'ps', dc)]
                row = qt * 4 + qs
                nc.tensor.matmul(
                    ps[:],
                    oT_norm[:, c, row * P:(row + 1) * P],
                    wo_sb[:, c, dc * 512:(dc + 1) * 512],
                    start=(c == 0), stop=(c == 1),
                )
                if c == 1:
                    o_sb = oproj_sb[qt]
                    dst = o_sb[:, qs, dc * 512:(dc + 1) * 512]
                    nc.vector.tensor_copy(dst, ps[:])
                    if dc == 1:
                        ch = 2 * qt + qs // 2
                        nc.sync.dma_start(
                            cc_in[ch][(qs % 2) * P:(qs % 2 + 1) * P, :],
                            o_sb[:, qs, :])

            def oproj_rs(ch):
                nc.gpsimd.collective_compute(
                    "ReduceScatter", mybir.AluOpType.add,
                    replica_groups=[[0, 1, 2, 3], [4, 5, 6, 7]],
                    ins=[cc_in[ch].opt()], outs=[cc_out[ch].opt()],
                )
                # out write rides the gpsimd queue behind its own collective;
                # nothing later depends on it, so it can never head-block
                nc.gpsimd.dma_start(out[ch * 64:(ch + 1) * 64, :],
                                    cc_out[ch][:])

            # filler schedule for (hp=1, qt): oproj(qt-1)'s 16 matmuls in
            # kb slots 4..15 (norm of qt-1 needs the early slots to finish)
            OP_STEPS = {kb: [] for kb in range(NSC)}
            for m in range(16):
                OP_STEPS[4 + m * 12 // 16].append(m)

            def oproj_filler(qt, kb):
                if qt == 0:
                    return
                for m in OP_STEPS[kb]:
                    oproj_mm(qt - 1, m)
                    if m == 7:
                        oproj_rs(2 * (qt - 1))
                if kb == NSC - 1:
                    oproj_rs(2 * (qt - 1) + 1)

            # ---- attention: head-pair outer, q-tile, stream k in pairs ----
            for hp in range(2):              # head pair (2hp, 2hp+1)
                for qt in range(NQT):
                    # both heads' unnormalized o^T accumulate in one PSUM
                    # bank: head j occupies partitions 64j..64j+63, so the
                    # two M=64 attnV matmuls run as concurrent column tiles
                    oT = ps_oT.tile([P, 512], fp32, tag="oT",
                                    name=f"oT_{hp}_{qt}")
                    # running sum of em-scaled exp tiles (softmax denominator)
                    acc = accp.tile([P, 2, 512], fp16, tag="acc",
                                    name=f"acc_{hp}_{qt}")

                    def attn_v(kb, at_kb):
                        for j in range(2):
                            h = 2 * hp + j
                            nc.tensor.matmul(
                                oT[64 * j:64 * (j + 1), :],
                                V_sb[:, kb, h * HD:(h + 1) * HD],
                                at_kb[:, j, :],
                                start=(kb == 0), stop=(kb == NSC - 1),
                            )

                    # attnV software-pipelined two steps behind the exp
                    # stream so the PE never waits on the current tile's exp
                    pend = []
                    for kb in range(NSC):
                        sT = ps_sT.tile([P, 2, 512], fp32, tag="sT")
                        # the two heads occupy partitions 0-63 / 64-127, so the
                        # two K=64 score matmuls run concurrently as row tiles
                        for j in range(2):
                            nc.tensor.matmul(
                                sT[:, j, :],
                                KT_rot[64 * j:64 * j + 64, hp, kb * P:(kb + 1) * P],
                                QT_rot[64 * j:64 * j + 64, hp,
                                       qt * 512:(qt + 1) * 512],
                                start=True, stop=True,
                            )
                        at = attp.tile([P, 2, 512], fp16, tag="attnT")
                        nc.scalar.activation(
                            at[:], sT[:], mybir.ActivationFunctionType.Exp,
                            scale=0.125)
                        if kb == 0:
                            nc.vector.tensor_scalar_mul(
                                acc[:], at[:], em_sb[:, 0:1])
                        else:
                            nc.vector.scalar_tensor_tensor(
                                acc[:], at[:], em_sb[:, kb:kb + 1], acc[:],
                                mybir.AluOpType.mult, mybir.AluOpType.add)
                        if hp == 0:
                            defer_step(16 * qt + kb)
                        else:
                            oproj_filler(qt, kb)
                        pend.append((kb, at))
                        if len(pend) > 2:
                            attn_v(*pend.pop(0))
                    for p in pend:
                        attn_v(*p)
                    # softmax denominators: ones^T @ acc per head, then
                    # approx-reciprocal and a partition-broadcast via DRAM
                    rcd = dram_rc.tile([2, 512], fp32, tag="rcd",
                                       name=f"rcd_{hp}_{qt}")
                    for j in range(2):
                        rsp = ps_proj.tile([1, 512], fp32, tag="proj",
                                           name=f"rsp_{hp}_{qt}_{j}")
                        nc.tensor.matmul(rsp[:], ones_sb[:], acc[:, j, :],
                                         start=True, stop=True)
                        rcj = work.tile([1, 512], fp32, tag="rcj",
                                        name=f"rcj_{hp}_{qt}_{j}")
                        nc.vector.reciprocal_approx_fast(rcj[:], rsp[:])
                        nc.sync.dma_start(rcd[j:j + 1, :], rcj[:])
                    # stage values off PSUM (frees the bank for the next tile)
                    st_un = stun.tile([P, 512], bf16, tag="st_un",
                                      name=f"st_un_{qt}_{hp}")
                    nc.vector.tensor_copy(st_un[:], oT[:])
                    # normalization chain (overlaps the next tile's attention)
                    rb128 = work.tile([P, 512], fp32, tag="rbcast",
                                      name=f"rb_{hp}_{qt}")
                    for j in range(2):
                        rsrc = rcd[j:j + 1, :]
                        nc.sync.dma_start(
                            rb128[64 * j:64 * (j + 1), :],
                            bass.AP(rsrc.tensor, rsrc.offset, [[0, HD], [1, 512]]))
                    stage = work.tile([P, 512], bf16, tag="stage",
                                      name=f"stage_{hp}_{qt}")
                    nc.vector.tensor_mul(stage[:], st_un[:], rb128[:])
                    nc.sync.dma_start(
                        oT_norm[:, hp, qt * 512:(qt + 1) * 512], stage[:])

            # ---- tail: last chunk's output projection + ReduceScatter ----
            for m in range(16):
                oproj_mm(NQT - 1, m)
                if m == 7:
                    oproj_rs(2 * (NQT - 1))
            oproj_rs(2 * (NQT - 1) + 1)

    nc.compile()
    return nc


def _host_prep(x, pos, mask, wq_kernel, wq_bias, wk_kernel, wk_bias,
               wv_kernel, wv_bias, wo_kernel, wo_bias):
    """Build per-core in_maps for the 8 cores."""
    perm = np.array([(j // 2) if j % 2 == 0 else (j // 2 + 32)
                     for j in range(HD)])
    half = HD // 2
    freqs = (10000.0 ** (-np.linspace(0.0, 1.0, half, endpoint=False))).astype(np.float64)

    bf = ml_dtypes.bfloat16
    in_maps = []
    for c in range(8):
        b, g = c // 4, c % 4
        H = list(range(HPC * g, HPC * g + HPC))

        theta = pos[b].astype(np.float64)[:, None] * freqs[None, :]  # [S, 32]
        cos = np.cos(theta).astype(np.float32)
        sin = np.sin(theta).astype(np.float32)
        trigA = np.empty((P, S), np.float32)
        trigB = np.empty((P, S), np.float32)
        for r in range(P):
            j = r % HD
            i = j // 2
            trigA[r] = cos[:, i]
            trigB[r] = (-sin[:, i]) if j % 2 == 0 else sin[:, i]

        def permute_w(wk_):  # [D, H, hd] -> [D, 4*64] with rope-pair row order
            wsel = wk_[:, H, :][:, :, perm]          # [D, 4, 64]
            return np.ascontiguousarray(wsel.reshape(DIM, HPC * HD))

        def permute_b(bias):  # [H, hd] -> [128, 2]
            bsel = bias[H][:, perm]                  # [4, 64]
            return np.ascontiguousarray(bsel.reshape(2, P).T)

        emv = np.exp(mask[b, 0, 0].astype(np.float32))  # [S]

        in_maps.append({
            "xT": np.ascontiguousarray(x[b].T).astype(bf),
            "wq": permute_w(wq_kernel).astype(bf),
            "wk": permute_w(wk_kernel).astype(bf),
            "wv": np.ascontiguousarray(
                wv_kernel[:, H, :].reshape(DIM, HPC * HD)).astype(bf),
            "wo": np.ascontiguousarray(
                wo_kernel[H].reshape(HPC * HD, DIM)).astype(bf),
            "trigA": trigA.astype(bf),
            "trigB": trigB.astype(bf),
            "qbias": permute_b(wq_bias),
            "kbias": permute_b(wk_bias),
            "em": np.ascontiguousarray(emv.reshape(NSC, P).T),
        })
    return in_maps


def kernel(x, pos, mask, wq_kernel, wq_bias, wk_kernel, wk_bias,
           wv_kernel, wv_bias, wo_kernel, wo_bias):
    x, pos, mask = np.asarray(x), np.asarray(pos), np.asarray(mask)
    wq_kernel, wq_bias = np.asarray(wq_kernel), np.asarray(wq_bias)
    wk_kernel, wk_bias = np.asarray(wk_kernel), np.asarray(wk_bias)
    wv_kernel, wv_bias = np.asarray(wv_kernel), np.asarray(wv_bias)
    wo_kernel, wo_bias = np.asarray(wo_kernel), np.asarray(wo_bias)
    if "nc" not in _CACHE:
        _CACHE["nc"] = _build()
    nc = _CACHE["nc"]

    in_maps = _host_prep(x, pos, mask, wq_kernel, wq_bias, wk_kernel, wk_bias,
                         wv_kernel, wv_bias, wo_kernel, wo_bias)
    res = bass_utils.run_bass_kernel_spmd(
        nc, in_maps, core_ids=list(range(8)))

    final_bias = (wo_bias.astype(np.float64)
                  + np.einsum("hd,hdo->o", wv_bias.astype(np.float64),
                              wo_kernel.astype(np.float64))).astype(np.float32)

    # core (b, g) returns rows [256*ch + 64*g, +64) as its out[64*ch:...]
    outs = []
    for b in range(B):
        full = np.empty((S, DIM), np.float32)
        for g in range(4):
            o = np.asarray(res.results[4 * b + g]["out"]).astype(np.float32)
            for ch in range(2 * NQT):
                full[256 * ch + 64 * g: 256 * ch + 64 * (g + 1)] = \
                    o[64 * ch: 64 * (ch + 1)]
        outs.append(full + final_bias[None, :])
    return np.stack(outs, axis=0)


# revision 10
# speedup vs baseline: 1.2557x; 1.2557x over previous
"""Multi-head attention (B=2, S=2048, D=1024, H=16, hd=64) with RoPE on 8 TRN2
NeuronCores.

Sharding: 2 batches x 4 head-groups. Core c handles batch c//4, heads
[4*(c%4), 4*(c%4)+4). Each core computes Q/K/V projections for its heads from
the full sequence, RoPE, then streamed attention per head-pair: scores as two
concurrent K=64 row-tiles, exp on the scalar engine (fp16 out), attnV as two
concurrent M=64 column-tiles of the PE array, and softmax row-sums from a DVE
accumulation of the exp tiles (scaled by exp(mask)) reduced by a ones-vector
matmul. The output projection is chunked by 512-row blocks as PE filler work
inside the next block's attention iterations, and each block is
ReduceScattered over the batch's 4-core group in two 256-row sub-chunks so
nearly all collective time overlaps attention. Core g of a group keeps rows
[256*ch + 64*g, +64) of sub-chunk ch; the host reassembles and adds the
(wo + wv@wo) bias.

Layout notes:
- x is uploaded pre-transposed (xT [D, S]) so it serves both as matmul rhs for
  Q^T/K^T production and as lhsT for V production.
- Q^T/K^T rows within each head are permuted to (d0,d32,d1,d33,...) so the
  RoPE partner lives in the adjacent partition; a stream_shuffle with the
  pair-swap mask plus two multiplies by host-precomputed cos/sin tables
  implements the rotation with all operands partition-aligned. The score
  matmul contracts over the permuted axis, which is permutation-invariant as
  long as Q and K share the ordering.
- The attention mask enters as exp(mask[k]): multiplied into V's rows for the
  numerator and into the exp tiles inside the row-sum accumulation, which is
  exact.
- Collectives live alone on the gpsimd queue; every DMA another engine later
  depends on is kept off that queue so a ReduceScatter wait can never head-block
  the attention pipeline.
"""

import numpy as np
import ml_dtypes

import concourse.bass as bass
import concourse.mybir as mybir
from concourse import bacc, bass_utils
import concourse.tile as tile

B, S, DIM, HEADS, HD = 2, 2048, 1024, 16, 64
HPC = HEADS // 4          # heads per core = 4
P = 128
NKC = DIM // P            # 8 contraction chunks for projections
NSC = S // P              # 16 sequence chunks of 128
NQT = S // 512            # 4 q tiles of 512
SQ = S // 4               # 512-row output slice per core
fp32 = mybir.dt.float32
bf16 = mybir.dt.bfloat16
fp16 = mybir.dt.float16

_CACHE = {}


def _build():
    nc = bacc.Bacc("TRN2", target_bir_lowering=False, debug=False, num_devices=8)

    xT = nc.dram_tensor("xT", [DIM, S], bf16, kind="ExternalInput")
    wq = nc.dram_tensor("wq", [DIM, HPC * HD], bf16, kind="ExternalInput")
    wk = nc.dram_tensor("wk", [DIM, HPC * HD], bf16, kind="ExternalInput")
    wv = nc.dram_tensor("wv", [DIM, HPC * HD], bf16, kind="ExternalInput")
    wo = nc.dram_tensor("wo", [HPC * HD, DIM], bf16, kind="ExternalInput")
    trigA = nc.dram_tensor("trigA", [P, S], bf16, kind="ExternalInput")
    trigB = nc.dram_tensor("trigB", [P, S], bf16, kind="ExternalInput")
    qbias = nc.dram_tensor("qbias", [P, 2], fp32, kind="ExternalInput")
    kbias = nc.dram_tensor("kbias", [P, 2], fp32, kind="ExternalInput")
    mb = nc.dram_tensor("mb", [P, NSC], fp32, kind="ExternalInput")
    out = nc.dram_tensor("out", [SQ, DIM], bf16, kind="ExternalOutput")

    SWAP_MASK = [i ^ 1 for i in range(32)]

    with tile.TileContext(nc) as tc:
        with (
            tc.tile_pool(name="const", bufs=1) as const,
            tc.tile_pool(name="work", bufs=3) as work,
            tc.tile_pool(name="attp", bufs=6) as attp,
            tc.tile_pool(name="accp", bufs=2) as accp,
            tc.tile_pool(name="stun", bufs=10) as stun,
            tc.tile_pool(name="ps_proj", bufs=2, space="PSUM") as ps_proj,
            tc.tile_pool(name="ps_sT", bufs=2, space="PSUM") as ps_sT,
            tc.tile_pool(name="ps_oT", bufs=2, space="PSUM") as ps_oT,
            tc.tile_pool(name="dram", bufs=1, space="DRAM") as dram,
            tc.tile_pool(name="dram_rc", bufs=3, space="DRAM") as dram_rc,
        ):
            # ---- load constants / inputs into SBUF ----
            # xT is split per kc chunk and spread over the sync and scalar
            # HWDGE queues (wq first on scalar so the kc=0 matmuls start
            # early); the gpsimd SWDGE queue carries only the later-needed
            # weights so the collectives it also hosts never gate anything.
            xT_sb = const.tile([P, NKC, S], bf16)
            xT_r = xT.rearrange("(c p) s -> p c s", p=P)
            wq_sb = const.tile([P, NKC, HPC * HD], bf16)
            wk_sb = const.tile([P, NKC, HPC * HD], bf16)
            wv_sb = const.tile([P, NKC, HPC * HD], bf16)
            wq_r = wq.rearrange("(c p) m -> p c m", p=P)
            wk_r = wk.rearrange("(c p) m -> p c m", p=P)
            wv_r = wv.rearrange("(c p) m -> p c m", p=P)
            nc.scalar.dma_start(wq_sb[:, 0:4, :], wq_r[:, 0:4, :])
            nc.sync.dma_start(xT_sb[:, 0, :], xT_r[:, 0, :])
            nc.scalar.dma_start(xT_sb[:, 1, :], xT_r[:, 1, :])
            nc.sync.dma_start(xT_sb[:, 2, :], xT_r[:, 2, :])
            nc.scalar.dma_start(wq_sb[:, 4:8, :], wq_r[:, 4:8, :])
            nc.sync.dma_start(xT_sb[:, 4, :], xT_r[:, 4, :])
            nc.scalar.dma_start(xT_sb[:, 3, :], xT_r[:, 3, :])
            nc.gpsimd.dma_start(wk_sb[:, 0:4, :], wk_r[:, 0:4, :])
            nc.sync.dma_start(xT_sb[:, 6, :], xT_r[:, 6, :])
            nc.scalar.dma_start(xT_sb[:, 5, :], xT_r[:, 5, :])
            nc.gpsimd.dma_start(wk_sb[:, 4:8, :], wk_r[:, 4:8, :])
            nc.scalar.dma_start(xT_sb[:, 7, :], xT_r[:, 7, :])
            qb_sb = const.tile([P, 2], fp32)
            nc.gpsimd.dma_start(qb_sb[:], qbias[:])
            kb_sb = const.tile([P, 2], fp32)
            nc.gpsimd.dma_start(kb_sb[:], kbias[:])
            mb_sb = const.tile([P, NSC], fp32)
            nc.gpsimd.dma_start(mb_sb[:], mb[:])
            trigA_sb = const.tile([P, S], bf16)
            nc.scalar.dma_start(trigA_sb[:], trigA[:])
            trigB_sb = const.tile([P, S], bf16)
            nc.sync.dma_start(trigB_sb[:], trigB[:])
            nc.gpsimd.dma_start(wv_sb[:, 0:4, :], wv_r[:, 0:4, :])
            nc.gpsimd.dma_start(wv_sb[:, 4:8, :], wv_r[:, 4:8, :])
            wo_sb = const.tile([P, 2, DIM], bf16)
            nc.scalar.dma_start(wo_sb[:], wo.rearrange("(c p) m -> p c m", p=P))

            # keep the PE busy from the preamble on: ~28 junk matmuls on a
            # zeroed tile warm the HAM clock gate (4/8 -> 8/8) before the
            # first real projection matmuls arrive
            wmm = work.tile([P, 512], bf16, tag="wmm", name="wmm")
            nc.vector.memset(wmm[:], 0.0)
            wps = ps_oT.tile([P, 512], fp32, tag="oT", name="warm_ps")
            for _ in range(28):
                nc.tensor.matmul(wps[:], wmm[:, 0:P], wmm[:],
                                 start=True, stop=True)

            warm_in = dram.tile([P, 4], fp32, name="warm_in")
            warm_out = dram.tile([P, 4], fp32, name="warm_out")
            wz = work.tile([P, 4], fp32, tag="wz", name="wz")
            nc.vector.memset(wz[:], 0.0)
            nc.gpsimd.dma_start(warm_in[:], wz[:])
            nc.gpsimd.collective_compute(
                "AllReduce", mybir.AluOpType.add,
                replica_groups=[[0, 1, 2, 3], [4, 5, 6, 7]],
                ins=[warm_in.opt()], outs=[warm_out.opt()],
            )

            QT_rot = const.tile([P, 2, S], bf16)   # heads 0,1 | 2,3 stacked
            KT_rot = const.tile([P, 2, S], bf16)
            V_sb = const.tile([P, NSC, HPC * HD], bf16)
            ones_sb = const.tile([P, 1], fp16)
            nc.vector.memset(ones_sb[:], 1.0)

            # ---- Q^T / K^T projections + RoPE ----
            # kc-outer so the stationary weight chunk is loaded once per kc
            # and streams the s-chunks. Only the cq=0 chunk (heads 0,1) is
            # produced up front; cq=1 matmuls are interleaved into head-pair
            # 0's attention iterations, where the PE has slack.
            def rope_chain(pss_sc, b_sb, dst, cq, sc, on_scalar):
                q_sb = work.tile([P, 512], bf16, tag="q_sb",
                                 name=f"q_sb_{cq}_{sc}")
                if on_scalar:
                    nc.scalar.add(q_sb[:], pss_sc[:], b_sb[:, cq:cq + 1])
                else:
                    nc.vector.tensor_scalar_add(
                        q_sb[:], pss_sc[:], b_sb[:, cq:cq + 1])
                q_sw = work.tile([P, 512], bf16, tag="q_sw",
                                 name=f"q_sw_{cq}_{sc}")
                nc.vector.stream_shuffle(q_sw[:], q_sb[:], SWAP_MASK)
                p1 = work.tile([P, 512], bf16, tag="p1", name=f"p1_{cq}_{sc}")
                nc.vector.tensor_mul(
                    p1[:], q_sb[:], trigA_sb[:, sc * 512:(sc + 1) * 512])
                p2 = work.tile([P, 512], bf16, tag="p2", name=f"p2_{cq}_{sc}")
                nc.vector.tensor_mul(
                    p2[:], q_sw[:], trigB_sb[:, sc * 512:(sc + 1) * 512])
                nc.vector.tensor_add(
                    dst[:, cq, sc * 512:(sc + 1) * 512], p1[:], p2[:])

            for w_sb, b_sb, dst in ((wq_sb, qb_sb, QT_rot), (wk_sb, kb_sb, KT_rot)):
                pss = []
                for sc in range(4):
                    pool = ps_proj if sc < 2 else ps_sT
                    tag = "proj" if sc < 2 else "sT"
                    pss.append(pool.tile([P, 512], fp32, tag=tag,
                                         name=f"pss_0_{sc}"))
                for kc in range(NKC):
                    for sc in range(4):
                        nc.tensor.matmul(
                            pss[sc][:],
                            w_sb[:, kc, 0:P],
                            xT_sb[:, kc, sc * 512:(sc + 1) * 512],
                            start=(kc == 0), stop=(kc == NKC - 1),
                        )
                for sc in range(4):
                    rope_chain(pss[sc], b_sb, dst, 0, sc, on_scalar=True)

            # deferred cq=1 projection groups: (weights, bias, dst, sc pair)
            defer_groups = [
                (wq_sb, qb_sb, QT_rot, (0, 1)),
                (wq_sb, qb_sb, QT_rot, (2, 3)),
                (wk_sb, kb_sb, KT_rot, (0, 1)),
                (wk_sb, kb_sb, KT_rot, (2, 3)),
            ]
            defer_pss = {}

            def defer_step(it):
                gi, m = divmod(it, 16)
                w_sb, b_sb, dst, scs = defer_groups[gi]
                if m == 0:
                    defer_pss[gi] = [
                        ps_proj.tile([P, 512], fp32, tag="proj",
                                     name=f"ip_{gi}_{k}")
                        for k in range(2)]
                kc, si = divmod(m, 2)
                nc.tensor.matmul(
                    defer_pss[gi][si][:],
                    w_sb[:, kc, P:2 * P],
                    xT_sb[:, kc, scs[si] * 512:(scs[si] + 1) * 512],
                    start=(kc == 0), stop=(kc == NKC - 1),
                )
                if m == 15:
                    for si in range(2):
                        rope_chain(defer_pss[gi][si], b_sb, dst, 1, scs[si],
                                   on_scalar=False)

            # ---- V projection (natural layout, scaled by exp(mask)) ----
            for sc in range(NSC):
                ps = ps_proj.tile([P, HPC * HD], fp32, tag="proj")
                for kc in range(NKC):
                    nc.tensor.matmul(
                        ps[:],
                        xT_sb[:, kc, sc * P:(sc + 1) * P],
                        wv_sb[:, kc, :],
                        start=(kc == 0), stop=(kc == NKC - 1),
                    )
                nc.vector.tensor_copy(V_sb[:, sc, :], ps[:])

            # ---- chunked output projection + ReduceScatter plumbing ----
            # oproj(qt) contracts the core's 4 heads for rows [512qt, +512)
            # and is emitted as PE filler work inside (hp=1, qt+1)'s
            # attention iterations; each 256-row half is ReduceScattered as
            # soon as its partials are staged, overlapping the remaining
            # attention. Sub-chunk ch rows land on group rank g as out rows
            # [64ch, +64) (host reassembles).
            oT_norm = const.tile([P, 2, S], bf16)   # normalized o^T, heads packed
            cc_in = [dram.tile([512, DIM], bf16, name=f"cc_in_{ch}")
                     for ch in range(NQT)]
            cc_out = [dram.tile([P, DIM], bf16, name=f"cc_out_{ch}")
                      for ch in range(NQT)]
            oproj_sb = {}

            def oproj_mm(qt, m):
                # one matmul of oproj(qt): m in [0,16)
                qs, r = divmod(m, 4)        # 4 row-chunks of 128
                dc, c = divmod(r, 2)        # 2 output halves x 2 contraction
                if m == 0:
                    oproj_sb[qt] = work.tile([P, 4, DIM], bf16, tag="o_sb",
                                             name=f"o_sb_{qt}")
                if c == 0:
                    oproj_sb[(qt, 'ps', dc)] = ps_proj.tile(
                        [P, 512], fp32, tag="proj", name=f"op_{qt}_{qs}_{dc}")
                ps = oproj_sb[(qt, 'ps', dc)]
                row = qt * 4 + qs
                nc.tensor.matmul(
                    ps[:],
                    oT_norm[:, c, row * P:(row + 1) * P],
                    wo_sb[:, c, dc * 512:(dc + 1) * 512],
                    start=(c == 0), stop=(c == 1),
                )
                if c == 1:
                    o_sb = oproj_sb[qt]
                    dst = o_sb[:, qs, dc * 512:(dc + 1) * 512]
                    nc.vector.tensor_copy(dst, ps[:])
                    if dc == 1:
                        nc.sync.dma_start(
                            cc_in[qt][qs * P:(qs + 1) * P, :], o_sb[:, qs, :])

            def oproj_rs(qt):
                nc.gpsimd.collective_compute(
                    "ReduceScatter", mybir.AluOpType.add,
                    replica_groups=[[0, 1, 2, 3], [4, 5, 6, 7]],
                    ins=[cc_in[qt].opt()], outs=[cc_out[qt].opt()],
                )
                # out write rides the gpsimd queue behind its own collective;
                # nothing later depends on it, so it can never head-block
                nc.gpsimd.dma_start(out[qt * P:(qt + 1) * P, :],
                                    cc_out[qt][:])

            # filler schedule for (hp=1, qt): oproj(qt-1)'s 16 matmuls in
            # kb slots 4..15 (norm of qt-1 needs the early slots to finish)
            OP_STEPS = {kb: [] for kb in range(NSC)}
            for m in range(16):
                OP_STEPS[4 + m * 12 // 16].append(m)

            def oproj_filler(qt, kb):
                if qt == 0:
                    return
                for m in OP_STEPS[kb]:
                    oproj_mm(qt - 1, m)
                if kb == NSC - 1:
                    oproj_rs(qt - 1)

            # ---- attention: head-pair outer, q-tile, stream k in pairs ----
            for hp in range(2):              # head pair (2hp, 2hp+1)
                for qt in range(NQT):
                    # both heads' unnormalized o^T accumulate in one PSUM
                    # bank: head j occupies partitions 64j..64j+63, so the
                    # two M=64 attnV matmuls run as concurrent column tiles
                    oT = ps_oT.tile([P, 512], fp32, tag="oT",
                                    name=f"oT_{hp}_{qt}")
                    # running sum of em-scaled exp tiles (softmax denominator)
                    acc = accp.tile([P, 2, 512], fp16, tag="acc",
                                    name=f"acc_{hp}_{qt}")

                    def attn_v(kb, at_kb):
                        for j in range(2):
                            h = 2 * hp + j
                            nc.tensor.matmul(
                                oT[64 * j:64 * (j + 1), :],
                                V_sb[:, kb, h * HD:(h + 1) * HD],
                                at_kb[:, j, :],
                                start=(kb == 0), stop=(kb == NSC - 1),
                            )

                    # attnV software-pipelined two steps behind the exp
                    # stream so the PE never waits on the current tile's exp
                    pend = []
                    for kb in range(NSC):
                        sT = ps_sT.tile([P, 2, 512], fp32, tag="sT")
                        # the two heads occupy partitions 0-63 / 64-127, so the
                        # two K=64 score matmuls run concurrently as row tiles
                        for j in range(2):
                            nc.tensor.matmul(
                                sT[:, j, :],
                                KT_rot[64 * j:64 * j + 64, hp, kb * P:(kb + 1) * P],
                                QT_rot[64 * j:64 * j + 64, hp,
                                       qt * 512:(qt + 1) * 512],
                                start=True, stop=True,
                            )
                        at = attp.tile([P, 2, 512], bf16, tag="attnT")
                        nc.scalar.activation(
                            at[:], sT[:], mybir.ActivationFunctionType.Exp,
                            bias=mb_sb[:, kb:kb + 1], scale=0.125)
                        if kb == 0:
                            nc.vector.tensor_copy(acc[:], at[:])
                        else:
                            nc.vector.tensor_add(acc[:], at[:], acc[:])
                        if hp == 0:
                            defer_step(16 * qt + kb)
                        else:
                            oproj_filler(qt, kb)
                        pend.append((kb, at))
                        if len(pend) > 2:
                            attn_v(*pend.pop(0))
                    for p in pend:
                        attn_v(*p)
                    # softmax denominators: ones^T @ acc per head, then
                    # approx-reciprocal and a partition-broadcast via DRAM
                    rcd = dram_rc.tile([2, 512], fp32, tag="rcd",
                                       name=f"rcd_{hp}_{qt}")
                    for j in range(2):
                        rsp = ps_proj.tile([1, 512], fp32, tag="proj",
                                           name=f"rsp_{hp}_{qt}_{j}")
                        nc.tensor.matmul(rsp[:], ones_sb[:], acc[:, j, :],
                                         start=True, stop=True)
                        rcj = work.tile([1, 512], fp32, tag="rcj",
                                        name=f"rcj_{hp}_{qt}_{j}")
                        nc.vector.reciprocal_approx_fast(rcj[:], rsp[:])
                        nc.sync.dma_start(rcd[j:j + 1, :], rcj[:])
                    # stage values off PSUM (frees the bank for the next tile)
                    st_un = stun.tile([P, 512], bf16, tag="st_un",
                                      name=f"st_un_{qt}_{hp}")
                    nc.vector.tensor_copy(st_un[:], oT[:])
                    # normalization chain (overlaps the next tile's attention)
                    rb128 = work.tile([P, 512], fp32, tag="rbcast",
                                      name=f"rb_{hp}_{qt}")
                    for j in range(2):
                        rsrc = rcd[j:j + 1, :]
                        nc.sync.dma_start(
                            rb128[64 * j:64 * (j + 1), :],
                            bass.AP(rsrc.tensor, rsrc.offset, [[0, HD], [1, 512]]))
                    stage = work.tile([P, 512], bf16, tag="stage",
                                      name=f"stage_{hp}_{qt}")
                    nc.vector.tensor_mul(stage[:], st_un[:], rb128[:])
                    nc.sync.dma_start(
                        oT_norm[:, hp, qt * 512:(qt + 1) * 512], stage[:])

            # ---- tail: last chunk's output projection + ReduceScatter ----
            for m in range(16):
                oproj_mm(NQT - 1, m)
            oproj_rs(NQT - 1)

    nc.compile()
    return nc


def _host_prep(x, pos, mask, wq_kernel, wq_bias, wk_kernel, wk_bias,
               wv_kernel, wv_bias, wo_kernel, wo_bias):
    """Build per-core in_maps for the 8 cores."""
    perm = np.array([(j // 2) if j % 2 == 0 else (j // 2 + 32)
                     for j in range(HD)])
    half = HD // 2
    freqs = (10000.0 ** (-np.linspace(0.0, 1.0, half, endpoint=False))).astype(np.float64)

    bf = ml_dtypes.bfloat16
    in_maps = []
    for c in range(8):
        b, g = c // 4, c % 4
        H = list(range(HPC * g, HPC * g + HPC))

        theta = pos[b].astype(np.float64)[:, None] * freqs[None, :]  # [S, 32]
        cos = np.cos(theta).astype(np.float32)
        sin = np.sin(theta).astype(np.float32)
        trigA = np.empty((P, S), np.float32)
        trigB = np.empty((P, S), np.float32)
        for r in range(P):
            j = r % HD
            i = j // 2
            trigA[r] = cos[:, i]
            trigB[r] = (-sin[:, i]) if j % 2 == 0 else sin[:, i]

        def permute_w(wk_):  # [D, H, hd] -> [D, 4*64] with rope-pair row order
            wsel = wk_[:, H, :][:, :, perm]          # [D, 4, 64]
            return np.ascontiguousarray(wsel.reshape(DIM, HPC * HD))

        def permute_b(bias):  # [H, hd] -> [128, 2]
            bsel = bias[H][:, perm]                  # [4, 64]
            return np.ascontiguousarray(bsel.reshape(2, P).T)

        maskv = mask[b, 0, 0].astype(np.float32)  # [S]

        in_maps.append({
            "xT": np.ascontiguousarray(x[b].T).astype(bf),
            "wq": permute_w(wq_kernel).astype(bf),
            "wk": permute_w(wk_kernel).astype(bf),
            "wv": np.ascontiguousarray(
                wv_kernel[:, H, :].reshape(DIM, HPC * HD)).astype(bf),
            "wo": np.ascontiguousarray(
                wo_kernel[H].reshape(HPC * HD, DIM)).astype(bf),
            "trigA": trigA.astype(bf),
            "trigB": trigB.astype(bf),
            "qbias": permute_b(wq_bias),
            "kbias": permute_b(wk_bias),
            "mb": np.ascontiguousarray(maskv.reshape(NSC, P).T),
        })
    return in_maps


def kernel(x, pos, mask, wq_kernel, wq_bias, wk_kernel, wk_bias,
           wv_kernel, wv_bias, wo_kernel, wo_bias):
    x, pos, mask = np.asarray(x), np.asarray(pos), np.asarray(mask)
    wq_kernel, wq_bias = np.asarray(wq_kernel), np.asarray(wq_bias)
    wk_kernel, wk_bias = np.asarray(wk_kernel), np.asarray(wk_bias)
    wv_kernel, wv_bias = np.asarray(wv_kernel), np.asarray(wv_bias)
    wo_kernel, wo_bias = np.asarray(wo_kernel), np.asarray(wo_bias)
    if "nc" not in _CACHE:
        _CACHE["nc"] = _build()
    nc = _CACHE["nc"]

    in_maps = _host_prep(x, pos, mask, wq_kernel, wq_bias, wk_kernel, wk_bias,
                         wv_kernel, wv_bias, wo_kernel, wo_bias)
    res = bass_utils.run_bass_kernel_spmd(
        nc, in_maps, core_ids=list(range(8)))

    final_bias = (wo_bias.astype(np.float64)
                  + np.einsum("hd,hdo->o", wv_bias.astype(np.float64),
                              wo_kernel.astype(np.float64))).astype(np.float32)

    # core (b, g) returns rows [512*qt + 128*g, +128) as its out[128*qt:...]
    outs = []
    for b in range(B):
        full = np.empty((S, DIM), np.float32)
        for g in range(4):
            o = np.asarray(res.results[4 * b + g]["out"]).astype(np.float32)
            for qt in range(NQT):
                full[512 * qt + P * g: 512 * qt + P * (g + 1)] = \
                    o[P * qt: P * (qt + 1)]
        outs.append(full + final_bias[None, :])
    return np.stack(outs, axis=0)
